# revision 34
# baseline (speedup 1.0000x reference)
"""MiniMHSA Trainium2 kernel: 8 NeuronCores, shard = (batch n, head-group).

Reference computes, per batch n:
  qkv = x @ W_qkv.T + b_qkv ; split into q,k,v heads (H=16, HD=64)
  scores = (q @ k.T) / sqrt(HD), masked keys -> -1e9, softmax, @ v
  out = attn_out @ W_out.T + b_out

Core c handles n = c//2 and head-group hg = c%2 (8 heads each).

Key optimizations over the naive dataflow:
  * Mask compaction: masked keys contribute exactly 0 to softmax, so k/v are
    only computed for the ~50% unmasked keys (host gathers x columns, pads to
    a multiple of 128). Scores/PV/k-proj/v-proj all shrink proportionally.
  * Head-pair packing: two heads share the 128 partitions everywhere
    (partition p = (h%2)*64 + dim). Out-projection contracts 4 chunks of 128
    instead of 8 of 64.
  * v-bias + out-bias folded on host: softmax weights sum to 1, so the v bias
    contributes bv @ W_out.T — a constant folded into an effective out bias.
  * bf16 x/weights/attention operands (PE cost identical, halves SBUF/DMA);
    f32r out-projection.
  * No max-subtraction softmax: exp(s + mask_bias) directly (scores are O(5));
    denominator via an appended ones-column in v.
  * Pipelined attention: S(kc+1) ahead of PV(kc); normalization of iteration
    i-1 emitted inside iteration i; leftover projection / out-projection
    groups interleaved as PE filler during exp-paced attention.
"""
import sys

sys.path.insert(0, '/opt/trn_rl_repo')


import numpy as np

_KERNEL_CACHE = {}


def _split_excess_waits(nc):
    """Walrus codegen reliably accepts only ONE sync wait per instruction
    (Matmult hard-fails at 2, Drain at 5). Tile's scheduler can attach more.
    Move excess waits onto preceding same-engine NOPs — semantically identical
    since engine queues execute in order."""
    from concourse import mybir

    for f in nc.m.functions:
        for blk in f.blocks:
            il = blk.instructions
            i = 0
            while i < len(il):
                inst = il[i]
                si = inst.sync_info
                waits = list(si.on_wait) if si is not None and si.on_wait else []
                if len(waits) > 1:
                    keep = waits[-1:]
                    excess = waits[:-1]
                    pos = i
                    for j, wcond in enumerate(excess):
                        nop = mybir.InstNoOp(name=f"{inst.name}-ws{j}", ins=[], outs=[])
                        nop.engine = inst.engine
                        nop.sync_info = mybir.SyncInfo(on_wait=[wcond], on_update=[])
                        il.insert(pos, nop)
                        pos += 1
                        i += 1
                    inst.sync_info = mybir.SyncInfo(
                        on_wait=keep,
                        on_update=list(si.on_update) if si.on_update else [],
                    )
                i += 1


def _col_chunks(total):
    """Split a multiple-of-128 column count into chunks of 256..512 (each a
    multiple of 128) so fp32r matmuls stay >= 256 moving columns."""
    out, rem = [], total
    while rem > 512:
        step = 512 if rem - 512 >= 256 else 384
        out.append(step)
        rem -= step
    out.append(rem)
    return out


def _build(cfg, waitsplit=True):
    import concourse.bass as bass
    import concourse.tile as tile
    from concourse import mybir

    F32 = mybir.dt.float32
    F32R = mybir.dt.float32r
    BF16 = mybir.dt.bfloat16
    AF = mybir.ActivationFunctionType
    MULT = mybir.AluOpType.mult
    ADD = mybir.AluOpType.add

    L, D, KPC = cfg["L"], cfg["D"], cfg["KPC"]
    HC, HD = cfg["HC"], cfg["HD"]
    DCH = D // 128            # contraction chunks for projections
    PAIRS = HC // 2
    KPAD = KPC * 128
    QH = L // 1024            # attention q hemis (1024 wide)
    DOUT = D
    DC = DOUT // 512
    KCH = _col_chunks(KPAD)
    KST = [0]
    for w in KCH:
        KST.append(KST[-1] + w)

    nc = bass.Bass()
    xT_d = nc.dram_tensor("xT", [D, L], BF16, kind="ExternalInput")
    xTk_d = nc.dram_tensor("xTk", [D, KPAD], BF16, kind="ExternalInput")
    wqk_d = nc.dram_tensor("wqk", [128, DCH, 1024], BF16, kind="ExternalInput")
    wv_d = nc.dram_tensor("wv", [128, DCH, 512], BF16, kind="ExternalInput")
    bqk_d = nc.dram_tensor("bqk", [128, 2 * PAIRS], F32, kind="ExternalInput")
    mb_d = nc.dram_tensor("mb", [128, KPC], F32, kind="ExternalInput")
    wo_d = nc.dram_tensor("wo", [128, PAIRS, DOUT], F32, kind="ExternalInput")
    bo_d = nc.dram_tensor("bo", [1, DOUT], F32, kind="ExternalInput")
    y_d = nc.dram_tensor("y", [L, DOUT], F32, kind="ExternalOutput")

    # partition_broadcast (gpsimd ISA) is rejected by the walrus codegen in
    # this container — keep the PE ones-matmul broadcast path.
    pbcast = cfg.get("PBCAST", False)
    with tile.TileContext(nc) as tc, \
         nc.allow_low_precision(reason="float32r/bf16 matmuls intended"):
        if pbcast:
            from concourse import library_config
            nc.gpsimd.load_library(library_config.attn)
        with tc.tile_pool(name="const", bufs=1) as const, \
             tc.tile_pool(name="big", bufs=1) as big, \
             tc.tile_pool(name="xp", bufs=1) as xp, \
             tc.tile_pool(name="workP", bufs=4) as workP, \
             tc.tile_pool(name="workS", bufs=2) as workS, \
             tc.tile_pool(name="psB", bufs=2, space="PSUM") as psB, \
             tc.tile_pool(name="psC", bufs=1, space="PSUM") as psC, \
             tc.tile_pool(name="psF", bufs=2, space="PSUM") as psF:

            # ---- constants / weights (pool DMA queue; k-weights first: the
            # first PE groups are k-proj and wait on them) ----
            wqk_r = const.tile([128, DCH, 1024], BF16, tag="wqk")
            nc.gpsimd.dma_start(out=wqk_r[:, :, 512:640], in_=wqk_d[:, :, 512:640])
            nc.gpsimd.dma_start(out=wqk_r[:, :, 640:768], in_=wqk_d[:, :, 640:768])
            nc.gpsimd.dma_start(out=wqk_r[:, :, 768:1024], in_=wqk_d[:, :, 768:1024])
            bqk_t = const.tile([128, 2 * PAIRS], F32)
            nc.gpsimd.dma_start(out=bqk_t, in_=bqk_d[:, :])
            mb_t = const.tile([128, KPC], F32)
            nc.gpsimd.dma_start(out=mb_t, in_=mb_d[:, :])
            wv_r = const.tile([128, DCH, 512], BF16, tag="wv")
            nc.gpsimd.dma_start(out=wv_r, in_=wv_d[:, :, :])
            nc.gpsimd.dma_start(out=wqk_r[:, :, 0:512], in_=wqk_d[:, :, 0:512])
            wo_r = const.tile([128, PAIRS, DOUT], F32R, tag="wo")
            nc.gpsimd.dma_start(out=wo_r, in_=wo_d[:, :, :])
            bo_t = const.tile([1, DOUT], F32R)
            nc.gpsimd.dma_start(out=bo_t, in_=bo_d[:, :])
            ones_f = const.tile([128, 1], F32)
            nc.vector.memset(ones_f, 1.0)
            ones_r = const.tile([1, 128], F32R)
            nc.vector.tensor_copy(out=ones_r, in_=ones_f[0:1, 0:1].broadcast_to([1, 128]))
            bo_bc = const.tile([128, DOUT], F32)

            # streamed activations (sync DMA queue; xtk per-chunk first so the
            # first k-proj group only waits ~2.4us)
            xtk = xp.tile([128, DCH, KPAD], BF16, tag="xtk")
            xt = xp.tile([128, DCH, L], BF16, tag="xt")
            xtk_re = xTk_d.rearrange("(c p) l -> p c l", p=128)
            dma_cols = []
            if KCH[0] >= 512:
                half = KCH[0] // 2 // 128 * 128
                dma_cols += [(0, half), (half, KCH[0] - half)]
            else:
                dma_cols.append((0, KCH[0]))
            dma_cols += [(KST[ci], KCH[ci]) for ci in range(1, len(KCH))]
            for c0, w in dma_cols:
                nc.sync.dma_start(out=xtk[:, :, c0:c0 + w], in_=xtk_re[:, :, c0:c0 + w])
            nc.sync.dma_start(out=xt, in_=xT_d.rearrange("(c p) l -> p c l", p=128))

            qT = big.tile([128, PAIRS, L], BF16, tag="qT")
            kT = big.tile([128, PAIRS, KPAD], BF16, tag="kT")
            vp = big.tile([128, KPC, HC, HD + 1], BF16, tag="vp")
            otn = big.tile([128, PAIRS, L], F32R, tag="otn")
            nc.vector.tensor_copy(
                out=vp[:, :, :, HD:HD + 1],
                in_=ones_f.unsqueeze(1).unsqueeze(1).broadcast_to([128, KPC, HC, 1]),
            )

            # ---- PE work-group emitters (each = one PSUM accumulation group) ----
            def kproj_cols(mc, c0, w):
                ps = psF.tile([128, w], F32, tag="f", name="kproj_ps")
                for k in range(DCH):
                    nc.tensor.matmul(
                        ps, wqk_r[:, k, 512 + mc * 128:512 + (mc + 1) * 128],
                        xtk[:, k, c0:c0 + w], start=(k == 0), stop=(k == DCH - 1))
                nc.vector.tensor_scalar_add(
                    out=kT[:, mc, c0:c0 + w], in0=ps,
                    scalar1=bqk_t[:, PAIRS + mc:PAIRS + mc + 1])

            def kproj_group(mc, ci):
                kproj_cols(mc, KST[ci], KCH[ci])

            def vproj_group(kcg):
                ps = psF.tile([128, 512], F32, tag="f", name="vproj_ps")
                for k in range(DCH):
                    nc.tensor.matmul(
                        ps, xtk[:, k, kcg * 128:(kcg + 1) * 128],
                        wv_r[:, k, :], start=(k == 0), stop=(k == DCH - 1))
                nc.vector.tensor_copy(
                    out=vp[:, kcg, :, 0:HD],
                    in_=ps.rearrange("p (h d) -> p h d", h=HC))

            def qproj_group(mc, lc):
                ps = psF.tile([128, 512], F32, tag="f", name="qproj_ps")
                for k in range(DCH):
                    nc.tensor.matmul(
                        ps, wqk_r[:, k, mc * 128:(mc + 1) * 128],
                        xt[:, k, lc * 512:(lc + 1) * 512],
                        start=(k == 0), stop=(k == DCH - 1))
                nc.vector.tensor_scalar_add(
                    out=qT[:, mc, lc * 512:(lc + 1) * 512], in0=ps,
                    scalar1=bqk_t[:, mc:mc + 1])

            def bo_group(s):
                ps = psF.tile([128, 512], F32, tag="f", name="bo_ps")
                nc.tensor.matmul(ps, ones_r[0:1, :], bo_t[0:1, s * 512:(s + 1) * 512],
                                 start=True, stop=True)
                nc.vector.tensor_copy(out=bo_bc[:, s * 512:(s + 1) * 512], in_=ps)

            def outproj_qt(qt, lo=0, part=None):
                # part="partial": accumulate pairs lo..PAIRS/2-1 into a bf16
                # slot carved from the (dead after proj) xtk tile.
                # part="final": pairs lo..PAIRS-1 plus the stored partial.
                hi = PAIRS // 2 if part == "partial" else PAIRS
                if part == "partial":
                    y01 = xtk[:, qt % DCH, 0:DOUT]
                else:
                    y_sb = workS.tile([128, DOUT], F32, tag="y", name="y_sb")
                for dc in range(DC):
                    y_ps = psF.tile([128, 512], F32, tag="f", name="y_ps")
                    for i2 in range(lo, hi):
                        nc.tensor.matmul(
                            y_ps, otn[:, i2, qt * 128:(qt + 1) * 128],
                            wo_r[:, i2, dc * 512:(dc + 1) * 512],
                            start=(i2 == lo), stop=(i2 == hi - 1))
                    if part == "partial":
                        nc.vector.tensor_tensor(
                            out=y01[:, dc * 512:(dc + 1) * 512], in0=y_ps,
                            in1=bo_bc[:, dc * 512:(dc + 1) * 512], op=ADD)
                        continue
                    addend = xtk[:, qt % DCH, dc * 512:(dc + 1) * 512]                         if part == "final" else bo_bc[:, dc * 512:(dc + 1) * 512]
                    nc.vector.tensor_tensor(
                        out=y_sb[:, dc * 512:(dc + 1) * 512],
                        in0=y_ps, in1=addend, op=ADD)
                    nc.sync.dma_start(
                        out=y_d[qt * 128:(qt + 1) * 128, dc * 512:(dc + 1) * 512],
                        in_=y_sb[:, dc * 512:(dc + 1) * 512])

            # ---- pre-attention: k proj pairs 0-1, v proj, q proj pair 0
            # first hemi. The rest is pulled into attention gaps. ----
            LC = L // 512
            for ci in range(len(KCH)):
                if ci == 0 and KCH[0] >= 512:
                    half = KCH[0] // 2 // 128 * 128
                    kproj_cols(0, 0, half)
                    kproj_cols(1, 0, half)
                    kproj_cols(0, half, KCH[0] - half)
                    kproj_cols(1, half, KCH[0] - half)
                else:
                    kproj_group(0, ci)
                    kproj_group(1, ci)
                for sub in range(KCH[ci] // 128):
                    vproj_group(KST[ci] // 128 + sub)
            for lc in range(LC // 2):
                qproj_group(0, lc)
            for s in range(DC):
                bo_group(s)

            # filler order is deadline-driven: q pair mc (first hemi) is needed
            # from iteration 2*mc; second-hemi q chunks from the qh=1 iters.
            fillers = []
            fillers.append(lambda: qproj_group(1, 0))
            fillers.append(lambda: qproj_group(1, 1))
            for mc in (2, 3):
                for ci in range(len(KCH)):
                    fillers.append(lambda mc=mc, ci=ci: kproj_group(mc, ci))
                for lc in range(LC // 2):
                    fillers.append(lambda mc=mc, lc=lc: qproj_group(mc, lc))
            for mc in range(PAIRS):
                for lc in range(LC // 2, LC):
                    fillers.append(lambda mc=mc, lc=lc: qproj_group(mc, lc))
            fillers.reverse()      # pop() order = emission order

            # redundant re-projections: harmless PE work (no otn dependency)
            # used to pad the qh=1 normalization window once real filler runs
            # dry. Only pairs whose qh=1 iterations already finished are safe.
            redo = []  # half-split norm made the qh1 padding unnecessary

            def pull_fill(n):
                for _ in range(n):
                    if fillers:
                        fillers.pop()()

            # ---- attention ----
            outproj_done = 0
            partial_qts = set()
            prev_norm = [None]

            for it in range(QH * HC):
                qh, h = it // HC, it % HC
                i, j = h // 2, h % 2
                base = j * 64
                q0 = qh * 1024
                # two half-width accumulators: the normalization chain of half
                # A completes ~1us before B, unblocking the next iteration's
                # first PV that much sooner (psC bufs=1, so slot reuse gates it)
                ots = [psC.tile([HD + 1, 512], F32, tag=f"ot{s}", name=f"ot{s}")
                       for s in range(2)]
                prev = [None]

                def emit_pv(kc, pT):
                    for s in range(2):
                        nc.tensor.matmul(
                            ots[s],
                            vp[:, kc, h, :], pT[:, s * 512:(s + 1) * 512],
                            start=(kc == 0), stop=(kc == KPC - 1))

                def pull_slot(kc):
                    nonlocal outproj_done
                    if qh == 0:
                        if kc in (1, 3):
                            pull_fill(1)
                        return
                    if kc == 1:
                        if fillers:
                            pull_fill(1)
                        else:
                            while redo:
                                mc, lc = redo[0]
                                if 2 * mc + 1 < it - HC:   # its readers finished
                                    redo.pop(0)
                                    qproj_group(mc, lc)
                                    break
                                else:
                                    break
                        return
                    behind = outproj_done < min(it - HC + 1, (L // 128) // 2)
                    if kc in (3, 6) and behind:
                        outproj_qt(outproj_done)
                        outproj_done += 1

                for kc in range(KPC):
                    st = psB.tile([128, 1024], F32, tag="st", name="st")
                    for s in range(2):
                        nc.tensor.matmul(
                            st[:, s * 512:(s + 1) * 512],
                            kT[base:base + 64, i, kc * 128:(kc + 1) * 128],
                            qT[base:base + 64, i, q0 + s * 512:q0 + (s + 1) * 512],
                            start=True, stop=True)
                    if kc == 1 and prev_norm[0] is not None:
                        prev_norm[0]()
                        prev_norm[0] = None
                    if kc in (1, 3, 6):
                        pull_slot(kc)
                    if kc >= 1:
                        emit_pv(kc - 1, prev[0])
                    pT = workP.tile([128, 1024], BF16, tag="pT", name="pT")
                    nc.scalar.activation(
                        out=pT, in_=st, func=AF.Exp,
                        bias=mb_t[:, kc:kc + 1], scale=1.0)
                    prev[0] = pT
                emit_pv(KPC - 1, prev[0])

                recips = []
                for s in range(2):
                    recip = workS.tile([1, 512], F32R, tag=f"recip{s}",
                                       name=f"recip{s}")
                    nc.vector.reciprocal(out=recip, in_=ots[s][HD:HD + 1, :])
                    recips.append(recip)

                def make_norm(ots=ots, recips=recips, base=base, i=i, q0=q0):
                    def _norm():
                        for s in range(2):
                            bc_ps = psB.tile([64, 512], F32, tag="st",
                                             name="bc_ps")
                            nc.tensor.matmul(
                                bc_ps, ones_r[0:1, 0:64], recips[s][0:1, :],
                                start=True, stop=True)
                            bc_sb = workS.tile([64, 512], F32, tag=f"bc{s}",
                                               name="bc_sb")
                            nc.scalar.copy(out=bc_sb, in_=bc_ps)
                            nc.vector.tensor_tensor(
                                out=otn[base:base + 64, i,
                                        q0 + s * 512:q0 + (s + 1) * 512],
                                in0=ots[s][0:HD, :], in1=bc_sb, op=MULT)
                    return _norm

                prev_norm[0] = make_norm()

            pull_fill(len(fillers))

            # ---- output projection (remaining q-rows) ----
            # first tail row is staged: its first pair-matmuls are emitted
            # BEFORE the final normalization (they only read completed otn
            # regions), filling the PE while that norm's DVE chain drains.
            tail = list(range(outproj_done, L // 128))
            staged = []

            def stage_qt(qtS, pool_, tag_):
                y_sbS = workS.tile([128, DOUT], F32, tag="y", name="y_sb")
                pssS = []
                for dc in range(DC):
                    y_ps = pool_.tile([128, 512], F32, tag=tag_, name="y_ps")
                    for i2 in range(PAIRS - 1):
                        nc.tensor.matmul(
                            y_ps, otn[:, i2, qtS * 128:(qtS + 1) * 128],
                            wo_r[:, i2, dc * 512:(dc + 1) * 512],
                            start=(i2 == 0), stop=False)
                    pssS.append(y_ps)
                staged.append((qtS, y_sbS, pssS))

            if tail:
                stage_qt(tail[0], psF, "f")
            if prev_norm[0] is not None:
                prev_norm[0]()
            for qtS, y_sbS, pssS in staged:
                tail.remove(qtS)
                for dc in range(DC):
                    nc.tensor.matmul(
                        pssS[dc], otn[:, PAIRS - 1, qtS * 128:(qtS + 1) * 128],
                        wo_r[:, PAIRS - 1, dc * 512:(dc + 1) * 512],
                        start=False, stop=True)
                    nc.vector.tensor_tensor(
                        out=y_sbS[:, dc * 512:(dc + 1) * 512],
                        in0=pssS[dc], in1=bo_bc[:, dc * 512:(dc + 1) * 512], op=ADD)
                    nc.sync.dma_start(
                        out=y_d[qtS * 128:(qtS + 1) * 128, dc * 512:(dc + 1) * 512],
                        in_=y_sbS[:, dc * 512:(dc + 1) * 512])
            for qt in tail:
                outproj_qt(qt)

    # split multi-waits (walrus allows 1 sync wait per instruction reliably)
    if waitsplit:
        _split_excess_waits(nc)
    return nc


def _prep_inputs(x, mask, W_qkv, b_qkv, W_out, b_out, cfg):
    """Build the 8 per-core input maps (host-side shuffles)."""
    import ml_dtypes
    BF = ml_dtypes.bfloat16

    L, D, KPC = cfg["L"], cfg["D"], cfg["KPC"]
    HC, HD = cfg["HC"], cfg["HD"]
    DV = HC * HD              # 512 qkv dims per head-group
    KPAD = KPC * 128
    N = x.shape[0]
    scale = np.float32(1.0 / np.sqrt(HD))
    Wt = np.ascontiguousarray(W_qkv.T).astype(np.float32)    # [D, 3D]
    WoT = np.ascontiguousarray(W_out.T).astype(np.float32)   # [D, D]
    DCH = D // 128
    PAIRS = HC // 2

    # head-pair permutation: chunk mc, col c -> head 2mc + c//64, dim c%64
    idx = np.empty((PAIRS, 128), np.int64)
    for mc in range(PAIRS):
        c = np.arange(128)
        idx[mc] = (2 * mc + c // 64) * 64 + (c % 64)
    idxf = idx.reshape(-1)

    per_hg = []
    for hg in range(2):
        qs, ks, vs = hg * DV, D + hg * DV, 2 * D + hg * DV
        wq = Wt[:, qs:qs + DV][:, idxf] * scale
        wk = Wt[:, ks:ks + DV][:, idxf]
        wqk = np.concatenate([wq, wk], axis=1)                    # [D, 1024]
        wqk = np.ascontiguousarray(
            wqk.reshape(DCH, 128, 2 * DV).transpose(1, 0, 2)).astype(BF)
        wv = Wt[:, vs:vs + DV].reshape(DCH, 128, DV)
        wv = np.ascontiguousarray(wv.transpose(1, 0, 2)).astype(BF)
        bq = b_qkv[qs:qs + DV][idxf] * scale
        bk = b_qkv[ks:ks + DV][idxf]
        bqk = np.stack(
            [bq[mc * 128:(mc + 1) * 128] for mc in range(PAIRS)]
            + [bk[mc * 128:(mc + 1) * 128] for mc in range(PAIRS)], axis=1)
        bqk = np.ascontiguousarray(bqk).astype(np.float32)        # [128, 2*PAIRS]
        WoT_blk = WoT[hg * DV:(hg + 1) * DV, :]                   # [512, D]
        wo = np.ascontiguousarray(
            np.stack([WoT_blk[idx[i], :] for i in range(PAIRS)], axis=1))  # [128,4,D]
        # v-bias folds through softmax (weights sum to 1): bv @ WoT_blk
        bv = b_qkv[vs:vs + DV].astype(np.float32)
        bo_eff = bv @ WoT_blk
        if hg == 0:
            bo_eff = bo_eff + b_out.astype(np.float32)
        bo_eff = np.ascontiguousarray(bo_eff[None, :]).astype(np.float32)
        per_hg.append(dict(wqk=wqk, wv=wv, bqk=bqk, wo=wo, bo=bo_eff))

    xTs, xTks, mbs = [], [], []
    for n in range(N):
        xTs.append(np.ascontiguousarray(x[n].T).astype(BF))
        kept = np.nonzero(~mask[n])[0]
        xk = np.zeros((KPAD, D), np.float32)
        xk[:len(kept)] = x[n][kept]
        xTks.append(np.ascontiguousarray(xk.T).astype(BF))
        mb = np.full(KPAD, -1e9, np.float32)
        mb[:len(kept)] = 0.0
        mbs.append(np.ascontiguousarray(mb.reshape(KPC, 128).T))

    in_maps = []
    for c in range(2 * N):
        n, hg = c // 2, c % 2
        d = dict(per_hg[hg])
        d.update(xT=xTs[n], xTk=xTks[n], mb=mbs[n])
        in_maps.append(d)
    return in_maps


def kernel(x, mask, W_qkv, b_qkv, W_out, b_out):
    from concourse.bass_utils import run_bass_kernel_spmd

    x = np.asarray(x, dtype=np.float32)
    mask = np.asarray(mask).astype(bool)
    N, L, D = x.shape
    H = 16
    HD = D // H
    kept_max = int((~mask).sum(axis=1).max())
    KPC = max(2, -(-kept_max // 128))
    cfg = {"L": L, "D": D, "HC": H // 2, "HD": HD, "KPC": KPC}

    key = (L, D, H, KPC)
    if key not in _KERNEL_CACHE:
        _KERNEL_CACHE[key] = _build(cfg)
    nc = _KERNEL_CACHE[key]

    in_maps = _prep_inputs(
        x, mask,
        np.asarray(W_qkv, np.float32), np.asarray(b_qkv, np.float32),
        np.asarray(W_out, np.float32), np.asarray(b_out, np.float32), cfg,
    )
    res = run_bass_kernel_spmd(nc, in_maps, list(range(2 * N)))
    out = np.empty((N, L, D), np.float32)
    for n in range(N):
        out[n] = res.results[2 * n]["y"] + res.results[2 * n + 1]["y"]
    return out


# revision 35
# speedup vs baseline: 1.0052x; 1.0052x over previous
"""MiniMHSA Trainium2 kernel: 8 NeuronCores, shard = (batch n, head-group).

Reference computes, per batch n:
  qkv = x @ W_qkv.T + b_qkv ; split into q,k,v heads (H=16, HD=64)
  scores = (q @ k.T) / sqrt(HD), masked keys -> -1e9, softmax, @ v
  out = attn_out @ W_out.T + b_out

Core c handles n = c//2 and head-group hg = c%2 (8 heads each).

Key optimizations over the naive dataflow:
  * Mask compaction: masked keys contribute exactly 0 to softmax, so k/v are
    only computed for the ~50% unmasked keys (host gathers x columns, pads to
    a multiple of 128). Scores/PV/k-proj/v-proj all shrink proportionally.
  * Head-pair packing: two heads share the 128 partitions everywhere
    (partition p = (h%2)*64 + dim). Out-projection contracts 4 chunks of 128
    instead of 8 of 64.
  * v-bias + out-bias folded on host: softmax weights sum to 1, so the v bias
    contributes bv @ W_out.T — a constant folded into an effective out bias.
  * bf16 x/weights/attention operands (PE cost identical, halves SBUF/DMA);
    f32r out-projection.
  * No max-subtraction softmax: exp(s + mask_bias) directly (scores are O(5));
    denominator via an appended ones-column in v.
  * Pipelined attention: S(kc+1) ahead of PV(kc); normalization of iteration
    i-1 emitted inside iteration i; leftover projection / out-projection
    groups interleaved as PE filler during exp-paced attention.
"""
import sys

sys.path.insert(0, '/opt/trn_rl_repo')


import numpy as np

_KERNEL_CACHE = {}


def _split_excess_waits(nc):
    """Walrus codegen reliably accepts only ONE sync wait per instruction
    (Matmult hard-fails at 2, Drain at 5). Tile's scheduler can attach more.
    Move excess waits onto preceding same-engine NOPs — semantically identical
    since engine queues execute in order."""
    from concourse import mybir

    for f in nc.m.functions:
        for blk in f.blocks:
            il = blk.instructions
            i = 0
            while i < len(il):
                inst = il[i]
                si = inst.sync_info
                waits = list(si.on_wait) if si is not None and si.on_wait else []
                if len(waits) > 1:
                    keep = waits[-1:]
                    excess = waits[:-1]
                    pos = i
                    for j, wcond in enumerate(excess):
                        nop = mybir.InstNoOp(name=f"{inst.name}-ws{j}", ins=[], outs=[])
                        nop.engine = inst.engine
                        nop.sync_info = mybir.SyncInfo(on_wait=[wcond], on_update=[])
                        il.insert(pos, nop)
                        pos += 1
                        i += 1
                    inst.sync_info = mybir.SyncInfo(
                        on_wait=keep,
                        on_update=list(si.on_update) if si.on_update else [],
                    )
                i += 1


def _col_chunks(total):
    """Split a multiple-of-128 column count into chunks of 256..512 (each a
    multiple of 128) so fp32r matmuls stay >= 256 moving columns."""
    out, rem = [], total
    while rem > 512:
        step = 512 if rem - 512 >= 256 else 384
        out.append(step)
        rem -= step
    out.append(rem)
    return out


def _build(cfg, waitsplit=True):
    import concourse.bass as bass
    import concourse.tile as tile
    from concourse import mybir

    F32 = mybir.dt.float32
    F32R = mybir.dt.float32r
    BF16 = mybir.dt.bfloat16
    AF = mybir.ActivationFunctionType
    MULT = mybir.AluOpType.mult
    ADD = mybir.AluOpType.add

    L, D, KPC = cfg["L"], cfg["D"], cfg["KPC"]
    HC, HD = cfg["HC"], cfg["HD"]
    DCH = D // 128            # contraction chunks for projections
    PAIRS = HC // 2
    KPAD = KPC * 128
    QH = L // 1024            # attention q hemis (1024 wide)
    DOUT = D
    DC = DOUT // 512
    KCH = _col_chunks(KPAD)
    KST = [0]
    for w in KCH:
        KST.append(KST[-1] + w)

    nc = bass.Bass()
    xT_d = nc.dram_tensor("xT", [D, L], BF16, kind="ExternalInput")
    xTk_d = nc.dram_tensor("xTk", [D, KPAD], BF16, kind="ExternalInput")
    wqk_d = nc.dram_tensor("wqk", [128, DCH, 1024], BF16, kind="ExternalInput")
    wv_d = nc.dram_tensor("wv", [128, DCH, 512], BF16, kind="ExternalInput")
    bqk_d = nc.dram_tensor("bqk", [128, 2 * PAIRS], F32, kind="ExternalInput")
    mb_d = nc.dram_tensor("mb", [128, KPC], F32, kind="ExternalInput")
    wo_d = nc.dram_tensor("wo", [128, PAIRS, DOUT], F32, kind="ExternalInput")
    bo_d = nc.dram_tensor("bo", [1, DOUT], F32, kind="ExternalInput")
    y_d = nc.dram_tensor("y", [L, DOUT], F32, kind="ExternalOutput")

    # partition_broadcast (gpsimd ISA) is rejected by the walrus codegen in
    # this container — keep the PE ones-matmul broadcast path.
    pbcast = cfg.get("PBCAST", False)
    with tile.TileContext(nc) as tc, \
         nc.allow_low_precision(reason="float32r/bf16 matmuls intended"):
        if pbcast:
            from concourse import library_config
            nc.gpsimd.load_library(library_config.attn)
        with tc.tile_pool(name="const", bufs=1) as const, \
             tc.tile_pool(name="big", bufs=1) as big, \
             tc.tile_pool(name="xp", bufs=1) as xp, \
             tc.tile_pool(name="workP", bufs=4) as workP, \
             tc.tile_pool(name="workS", bufs=2) as workS, \
             tc.tile_pool(name="psB", bufs=2, space="PSUM") as psB, \
             tc.tile_pool(name="psC", bufs=1, space="PSUM") as psC, \
             tc.tile_pool(name="psF", bufs=2, space="PSUM") as psF:

            # ---- constants / weights (pool DMA queue; k-weights first: the
            # first PE groups are k-proj and wait on them) ----
            wqk_r = const.tile([128, DCH, 1024], BF16, tag="wqk")
            nc.gpsimd.dma_start(out=wqk_r[:, :, 512:768], in_=wqk_d[:, :, 512:768])
            nc.gpsimd.dma_start(out=wqk_r[:, :, 768:1024], in_=wqk_d[:, :, 768:1024])
            bqk_t = const.tile([128, 2 * PAIRS], F32)
            nc.gpsimd.dma_start(out=bqk_t, in_=bqk_d[:, :])
            mb_t = const.tile([128, KPC], F32)
            nc.gpsimd.dma_start(out=mb_t, in_=mb_d[:, :])
            wv_r = const.tile([128, DCH, 512], BF16, tag="wv")
            nc.gpsimd.dma_start(out=wv_r, in_=wv_d[:, :, :])
            nc.gpsimd.dma_start(out=wqk_r[:, :, 0:512], in_=wqk_d[:, :, 0:512])
            wo_r = const.tile([128, PAIRS, DOUT], F32R, tag="wo")
            nc.gpsimd.dma_start(out=wo_r, in_=wo_d[:, :, :])
            bo_t = const.tile([1, DOUT], F32R)
            nc.gpsimd.dma_start(out=bo_t, in_=bo_d[:, :])
            ones_f = const.tile([128, 1], F32)
            nc.vector.memset(ones_f, 1.0)
            ones_r = const.tile([1, 128], F32R)
            nc.vector.tensor_copy(out=ones_r, in_=ones_f[0:1, 0:1].broadcast_to([1, 128]))
            bo_bc = const.tile([128, DOUT], F32)

            # streamed activations (sync DMA queue; xtk per-chunk first so the
            # first k-proj group only waits ~2.4us)
            xtk = xp.tile([128, DCH, KPAD], BF16, tag="xtk")
            xt = xp.tile([128, DCH, L], BF16, tag="xt")
            xtk_re = xTk_d.rearrange("(c p) l -> p c l", p=128)
            dma_cols = []
            if KCH[0] >= 512:
                half = KCH[0] // 2 // 128 * 128
                dma_cols += [(0, half), (half, KCH[0] - half)]
            else:
                dma_cols.append((0, KCH[0]))
            dma_cols += [(KST[ci], KCH[ci]) for ci in range(1, len(KCH))]
            for c0, w in dma_cols:
                nc.sync.dma_start(out=xtk[:, :, c0:c0 + w], in_=xtk_re[:, :, c0:c0 + w])
            nc.sync.dma_start(out=xt, in_=xT_d.rearrange("(c p) l -> p c l", p=128))

            qT = big.tile([128, PAIRS, L], BF16, tag="qT")
            kT = big.tile([128, PAIRS, KPAD], BF16, tag="kT")
            vp = big.tile([128, KPC, HC, HD + 1], BF16, tag="vp")
            otn = big.tile([128, PAIRS, L], F32R, tag="otn")
            nc.vector.tensor_copy(
                out=vp[:, :, :, HD:HD + 1],
                in_=ones_f.unsqueeze(1).unsqueeze(1).broadcast_to([128, KPC, HC, 1]),
            )

            # ---- PE work-group emitters (each = one PSUM accumulation group) ----
            def kproj_cols(mc, c0, w):
                ps = psF.tile([128, w], F32, tag="f", name="kproj_ps")
                for k in range(DCH):
                    nc.tensor.matmul(
                        ps, wqk_r[:, k, 512 + mc * 128:512 + (mc + 1) * 128],
                        xtk[:, k, c0:c0 + w], start=(k == 0), stop=(k == DCH - 1))
                nc.vector.tensor_scalar_add(
                    out=kT[:, mc, c0:c0 + w], in0=ps,
                    scalar1=bqk_t[:, PAIRS + mc:PAIRS + mc + 1])

            def kproj_group(mc, ci):
                kproj_cols(mc, KST[ci], KCH[ci])

            def vproj_group(kcg):
                ps = psF.tile([128, 512], F32, tag="f", name="vproj_ps")
                for k in range(DCH):
                    nc.tensor.matmul(
                        ps, xtk[:, k, kcg * 128:(kcg + 1) * 128],
                        wv_r[:, k, :], start=(k == 0), stop=(k == DCH - 1))
                nc.vector.tensor_copy(
                    out=vp[:, kcg, :, 0:HD],
                    in_=ps.rearrange("p (h d) -> p h d", h=HC))

            def qproj_group(mc, lc):
                ps = psF.tile([128, 512], F32, tag="f", name="qproj_ps")
                for k in range(DCH):
                    nc.tensor.matmul(
                        ps, wqk_r[:, k, mc * 128:(mc + 1) * 128],
                        xt[:, k, lc * 512:(lc + 1) * 512],
                        start=(k == 0), stop=(k == DCH - 1))
                nc.vector.tensor_scalar_add(
                    out=qT[:, mc, lc * 512:(lc + 1) * 512], in0=ps,
                    scalar1=bqk_t[:, mc:mc + 1])

            def bo_group(s):
                ps = psF.tile([128, 512], F32, tag="f", name="bo_ps")
                nc.tensor.matmul(ps, ones_r[0:1, :], bo_t[0:1, s * 512:(s + 1) * 512],
                                 start=True, stop=True)
                nc.vector.tensor_copy(out=bo_bc[:, s * 512:(s + 1) * 512], in_=ps)

            def outproj_qt(qt, lo=0, part=None):
                # part="partial": accumulate pairs lo..PAIRS/2-1 into a bf16
                # slot carved from the (dead after proj) xtk tile.
                # part="final": pairs lo..PAIRS-1 plus the stored partial.
                hi = PAIRS // 2 if part == "partial" else PAIRS
                if part == "partial":
                    y01 = xtk[:, qt % DCH, 0:DOUT]
                else:
                    y_sb = workS.tile([128, DOUT], F32, tag="y", name="y_sb")
                for dc in range(DC):
                    y_ps = psF.tile([128, 512], F32, tag="f", name="y_ps")
                    for i2 in range(lo, hi):
                        nc.tensor.matmul(
                            y_ps, otn[:, i2, qt * 128:(qt + 1) * 128],
                            wo_r[:, i2, dc * 512:(dc + 1) * 512],
                            start=(i2 == lo), stop=(i2 == hi - 1))
                    if part == "partial":
                        nc.vector.tensor_tensor(
                            out=y01[:, dc * 512:(dc + 1) * 512], in0=y_ps,
                            in1=bo_bc[:, dc * 512:(dc + 1) * 512], op=ADD)
                        continue
                    addend = xtk[:, qt % DCH, dc * 512:(dc + 1) * 512]                         if part == "final" else bo_bc[:, dc * 512:(dc + 1) * 512]
                    nc.vector.tensor_tensor(
                        out=y_sb[:, dc * 512:(dc + 1) * 512],
                        in0=y_ps, in1=addend, op=ADD)
                    nc.sync.dma_start(
                        out=y_d[qt * 128:(qt + 1) * 128, dc * 512:(dc + 1) * 512],
                        in_=y_sb[:, dc * 512:(dc + 1) * 512])

            # ---- pre-attention: k proj pairs 0-1, v proj, q proj pair 0
            # first hemi. The rest is pulled into attention gaps. ----
            LC = L // 512
            for ci in range(len(KCH)):
                if ci == 0 and KCH[0] >= 512:
                    half = KCH[0] // 2 // 128 * 128
                    kproj_cols(0, 0, half)
                    kproj_cols(1, 0, half)
                    kproj_cols(0, half, KCH[0] - half)
                    kproj_cols(1, half, KCH[0] - half)
                else:
                    kproj_group(0, ci)
                    kproj_group(1, ci)
                for sub in range(KCH[ci] // 128):
                    vproj_group(KST[ci] // 128 + sub)
            for lc in range(LC // 2):
                qproj_group(0, lc)
            for s in range(DC):
                bo_group(s)

            # filler order is deadline-driven: q pair mc (first hemi) is needed
            # from iteration 2*mc; second-hemi q chunks from the qh=1 iters.
            fillers = []
            fillers.append(lambda: qproj_group(1, 0))
            fillers.append(lambda: qproj_group(1, 1))
            for mc in (2, 3):
                for ci in range(len(KCH)):
                    fillers.append(lambda mc=mc, ci=ci: kproj_group(mc, ci))
                for lc in range(LC // 2):
                    fillers.append(lambda mc=mc, lc=lc: qproj_group(mc, lc))
            for mc in range(PAIRS):
                for lc in range(LC // 2, LC):
                    fillers.append(lambda mc=mc, lc=lc: qproj_group(mc, lc))
            fillers.reverse()      # pop() order = emission order

            # redundant re-projections: harmless PE work (no otn dependency)
            # used to pad the qh=1 normalization window once real filler runs
            # dry. Only pairs whose qh=1 iterations already finished are safe.
            redo = []  # half-split norm made the qh1 padding unnecessary

            def pull_fill(n):
                for _ in range(n):
                    if fillers:
                        fillers.pop()()

            # ---- attention ----
            outproj_done = 0
            partial_qts = set()
            prev_norm = [None]

            for it in range(QH * HC):
                qh, h = it // HC, it % HC
                i, j = h // 2, h % 2
                base = j * 64
                q0 = qh * 1024
                # two half-width accumulators: the normalization chain of half
                # A completes ~1us before B, unblocking the next iteration's
                # first PV that much sooner (psC bufs=1, so slot reuse gates it)
                ots = [psC.tile([HD + 1, 512], F32, tag=f"ot{s}", name=f"ot{s}")
                       for s in range(2)]
                prev = [None]

                def emit_pv(kc, pT):
                    for s in range(2):
                        nc.tensor.matmul(
                            ots[s],
                            vp[:, kc, h, :], pT[:, s * 512:(s + 1) * 512],
                            start=(kc == 0), stop=(kc == KPC - 1))

                def pull_slot(kc):
                    nonlocal outproj_done
                    if qh == 0:
                        if kc in (1, 3):
                            pull_fill(1)
                        return
                    if kc == 1:
                        if fillers:
                            pull_fill(1)
                        else:
                            while redo:
                                mc, lc = redo[0]
                                if 2 * mc + 1 < it - HC:   # its readers finished
                                    redo.pop(0)
                                    qproj_group(mc, lc)
                                    break
                                else:
                                    break
                        return
                    behind = outproj_done < min(it - HC + 1, (L // 128) // 2)
                    if kc in (3, 6) and behind:
                        outproj_qt(outproj_done)
                        outproj_done += 1

                for kc in range(KPC):
                    st = psB.tile([128, 1024], F32, tag="st", name="st")
                    for s in range(2):
                        nc.tensor.matmul(
                            st[:, s * 512:(s + 1) * 512],
                            kT[base:base + 64, i, kc * 128:(kc + 1) * 128],
                            qT[base:base + 64, i, q0 + s * 512:q0 + (s + 1) * 512],
                            start=True, stop=True)
                    if kc == 1 and prev_norm[0] is not None:
                        prev_norm[0]()
                        prev_norm[0] = None
                    if kc in (1, 3, 6):
                        pull_slot(kc)
                    if kc >= 1:
                        emit_pv(kc - 1, prev[0])
                    pT = workP.tile([128, 1024], BF16, tag="pT", name="pT")
                    nc.scalar.activation(
                        out=pT, in_=st, func=AF.Exp,
                        bias=mb_t[:, kc:kc + 1], scale=1.0)
                    prev[0] = pT
                emit_pv(KPC - 1, prev[0])

                recips = []
                for s in range(2):
                    recip = workS.tile([1, 512], F32R, tag=f"recip{s}",
                                       name=f"recip{s}")
                    nc.vector.reciprocal(out=recip, in_=ots[s][HD:HD + 1, :])
                    recips.append(recip)

                def make_norm(ots=ots, recips=recips, base=base, i=i, q0=q0):
                    def _norm():
                        for s in range(2):
                            bc_ps = psB.tile([64, 512], F32, tag="st",
                                             name="bc_ps")
                            nc.tensor.matmul(
                                bc_ps, ones_r[0:1, 0:64], recips[s][0:1, :],
                                start=True, stop=True)
                            bc_sb = workS.tile([64, 512], F32, tag=f"bc{s}",
                                               name="bc_sb")
                            nc.scalar.copy(out=bc_sb, in_=bc_ps)
                            nc.vector.tensor_tensor(
                                out=otn[base:base + 64, i,
                                        q0 + s * 512:q0 + (s + 1) * 512],
                                in0=ots[s][0:HD, :], in1=bc_sb, op=MULT)
                    return _norm

                prev_norm[0] = make_norm()

            pull_fill(len(fillers))

            # ---- output projection (remaining q-rows) ----
            # first tail row is staged: its first pair-matmuls are emitted
            # BEFORE the final normalization (they only read completed otn
            # regions), filling the PE while that norm's DVE chain drains.
            tail = list(range(outproj_done, L // 128))
            staged = []

            def stage_qt(qtS, pool_, tag_):
                y_sbS = workS.tile([128, DOUT], F32, tag="y", name="y_sb")
                pssS = []
                for dc in range(DC):
                    y_ps = pool_.tile([128, 512], F32, tag=tag_, name="y_ps")
                    for i2 in range(PAIRS - 1):
                        nc.tensor.matmul(
                            y_ps, otn[:, i2, qtS * 128:(qtS + 1) * 128],
                            wo_r[:, i2, dc * 512:(dc + 1) * 512],
                            start=(i2 == 0), stop=False)
                    pssS.append(y_ps)
                staged.append((qtS, y_sbS, pssS))

            if tail:
                stage_qt(tail[0], psF, "f")
            if prev_norm[0] is not None:
                prev_norm[0]()
            if len(tail) > 1:
                stage_qt(tail[1], psB, "st")
            for qtS, y_sbS, pssS in staged:
                tail.remove(qtS)
                for dc in range(DC):
                    nc.tensor.matmul(
                        pssS[dc], otn[:, PAIRS - 1, qtS * 128:(qtS + 1) * 128],
                        wo_r[:, PAIRS - 1, dc * 512:(dc + 1) * 512],
                        start=False, stop=True)
                    nc.vector.tensor_tensor(
                        out=y_sbS[:, dc * 512:(dc + 1) * 512],
                        in0=pssS[dc], in1=bo_bc[:, dc * 512:(dc + 1) * 512], op=ADD)
                    nc.sync.dma_start(
                        out=y_d[qtS * 128:(qtS + 1) * 128, dc * 512:(dc + 1) * 512],
                        in_=y_sbS[:, dc * 512:(dc + 1) * 512])
            for qt in tail:
                outproj_qt(qt)

    # split multi-waits (walrus allows 1 sync wait per instruction reliably)
    if waitsplit:
        _split_excess_waits(nc)
    return nc


def _prep_inputs(x, mask, W_qkv, b_qkv, W_out, b_out, cfg):
    """Build the 8 per-core input maps (host-side shuffles)."""
    import ml_dtypes
    BF = ml_dtypes.bfloat16

    L, D, KPC = cfg["L"], cfg["D"], cfg["KPC"]
    HC, HD = cfg["HC"], cfg["HD"]
    DV = HC * HD              # 512 qkv dims per head-group
    KPAD = KPC * 128
    N = x.shape[0]
    scale = np.float32(1.0 / np.sqrt(HD))
    Wt = np.ascontiguousarray(W_qkv.T).astype(np.float32)    # [D, 3D]
    WoT = np.ascontiguousarray(W_out.T).astype(np.float32)   # [D, D]
    DCH = D // 128
    PAIRS = HC // 2

    # head-pair permutation: chunk mc, col c -> head 2mc + c//64, dim c%64
    idx = np.empty((PAIRS, 128), np.int64)
    for mc in range(PAIRS):
        c = np.arange(128)
        idx[mc] = (2 * mc + c // 64) * 64 + (c % 64)
    idxf = idx.reshape(-1)

    per_hg = []
    for hg in range(2):
        qs, ks, vs = hg * DV, D + hg * DV, 2 * D + hg * DV
        wq = Wt[:, qs:qs + DV][:, idxf] * scale
        wk = Wt[:, ks:ks + DV][:, idxf]
        wqk = np.concatenate([wq, wk], axis=1)                    # [D, 1024]
        wqk = np.ascontiguousarray(
            wqk.reshape(DCH, 128, 2 * DV).transpose(1, 0, 2)).astype(BF)
        wv = Wt[:, vs:vs + DV].reshape(DCH, 128, DV)
        wv = np.ascontiguousarray(wv.transpose(1, 0, 2)).astype(BF)
        bq = b_qkv[qs:qs + DV][idxf] * scale
        bk = b_qkv[ks:ks + DV][idxf]
        bqk = np.stack(
            [bq[mc * 128:(mc + 1) * 128] for mc in range(PAIRS)]
            + [bk[mc * 128:(mc + 1) * 128] for mc in range(PAIRS)], axis=1)
        bqk = np.ascontiguousarray(bqk).astype(np.float32)        # [128, 2*PAIRS]
        WoT_blk = WoT[hg * DV:(hg + 1) * DV, :]                   # [512, D]
        wo = np.ascontiguousarray(
            np.stack([WoT_blk[idx[i], :] for i in range(PAIRS)], axis=1))  # [128,4,D]
        # v-bias folds through softmax (weights sum to 1): bv @ WoT_blk
        bv = b_qkv[vs:vs + DV].astype(np.float32)
        bo_eff = bv @ WoT_blk
        if hg == 0:
            bo_eff = bo_eff + b_out.astype(np.float32)
        bo_eff = np.ascontiguousarray(bo_eff[None, :]).astype(np.float32)
        per_hg.append(dict(wqk=wqk, wv=wv, bqk=bqk, wo=wo, bo=bo_eff))

    xTs, xTks, mbs = [], [], []
    for n in range(N):
        xTs.append(np.ascontiguousarray(x[n].T).astype(BF))
        kept = np.nonzero(~mask[n])[0]
        xk = np.zeros((KPAD, D), np.float32)
        xk[:len(kept)] = x[n][kept]
        xTks.append(np.ascontiguousarray(xk.T).astype(BF))
        mb = np.full(KPAD, -1e9, np.float32)
        mb[:len(kept)] = 0.0
        mbs.append(np.ascontiguousarray(mb.reshape(KPC, 128).T))

    in_maps = []
    for c in range(2 * N):
        n, hg = c // 2, c % 2
        d = dict(per_hg[hg])
        d.update(xT=xTs[n], xTk=xTks[n], mb=mbs[n])
        in_maps.append(d)
    return in_maps


def kernel(x, mask, W_qkv, b_qkv, W_out, b_out):
    from concourse.bass_utils import run_bass_kernel_spmd

    x = np.asarray(x, dtype=np.float32)
    mask = np.asarray(mask).astype(bool)
    N, L, D = x.shape
    H = 16
    HD = D // H
    kept_max = int((~mask).sum(axis=1).max())
    KPC = max(2, -(-kept_max // 128))
    cfg = {"L": L, "D": D, "HC": H // 2, "HD": HD, "KPC": KPC}

    key = (L, D, H, KPC)
    if key not in _KERNEL_CACHE:
        _KERNEL_CACHE[key] = _build(cfg)
    nc = _KERNEL_CACHE[key]

    in_maps = _prep_inputs(
        x, mask,
        np.asarray(W_qkv, np.float32), np.asarray(b_qkv, np.float32),
        np.asarray(W_out, np.float32), np.asarray(b_out, np.float32), cfg,
    )
    res = run_bass_kernel_spmd(nc, in_maps, list(range(2 * N)))
    out = np.empty((N, L, D), np.float32)
    for n in range(N):
        out[n] = res.results[2 * n]["y"] + res.results[2 * n + 1]["y"]
    return out
